# revision 26
# baseline (speedup 1.0000x reference)
"""CBOW negative-sampling loss on 8 TRN2 NeuronCores.

Data-parallel: batch dim (16384) sharded 8 ways (2048 rows/core).

The gather (the memory-bound core of this problem) uses the bulk
InstDMAGatherAnt extended instruction (~900 rows per instruction,
rotated across the 4 SWDGE queues so all four Q7 core pairs generate
DMA descriptors in parallel) instead of per-row indirect DMAs, which
cost ~1us of serialized descriptor-generation per 128 rows.
dma_gather takes int16 indices (< 32768), but VOCAB=100000 — so the
host dedups and relabels each half-core's referenced rows into a
compacted table upload with static per-half slabs:

  - per core, per half (1024 batch rows), per table: the referenced
    vocab rows are uniqued (sorted) and relabeled 0..U-1; the table
    slab uploaded to the device is table[uniq] padded to a static cap
    (cap = the draw count, an absolute bound on U, ~20.5k < 32768).
  - gather indices are the relabeled ids, wrapped in dma_gather's
    [16, n/16] layout and replicated across all 128 partitions.

Per tile of 128 rows (one batch row per partition):
  - 3 dma_gathers: 20 context rows/partition -> ctx_g [128, 20, 128]
  - 3 dma_gathers: 20 negatives + 1 target   -> ng_g  [128, 21, 128]
  - ACT copies ctx_g to bf16 (exact-identity matmul inputs)
  - PE: 20 PSUM-accumulating bf16 identity matmuls -> ctx_sum (fp32)
  - DVE: broadcast-mult (in1 straight from PSUM) + reduce over EMB
    -> scores [128, 21]; clip to [-10, 10] in one chained min/max op
  - ACT Exp: negs (softplus(+s)) and target with scale=-1
    (softplus(-s) == -log_sigmoid(s)) into slices of exp_all
Final: one ACT Ln(1 + x) with accum_out over all 16*21 values (= sum
of softplus terms per partition), then a ones-vector matmul on the PE
reduces across partitions.  Host sums the 8 partials and divides by B.
"""

import os
import numpy as np

VOCAB, EMB = 100000, 128
B, C, N = 16384, 20, 20
NCORES = 8
RPC = B // NCORES  # 2048 rows per core
P = 128
TILES = RPC // P  # 16
N1 = N + 1  # negatives + target
HALVES = 2
TPH = TILES // HALVES  # tiles per half
CTX_CAP = TPH * P * C  # 20480 — absolute bound on unique rows per half
NG_CAP = TPH * P * N1  # 21504
CTX_W = P * C // 16  # 160 wrapped idx cols per tile
NG_W = P * N1 // 16  # 168

_compiled = None
last_results = None
import ml_dtypes as _mld

_IDENT = np.eye(P, dtype=_mld.bfloat16)


def _build(tiles=TILES, nqueues=4):
    import concourse.bacc as bacc
    import concourse.tile as tile
    from concourse import bass, library_config, mybir

    f32 = mybir.dt.float32
    bf16 = mybir.dt.bfloat16
    i16 = mybir.dt.int16
    AX = mybir.AxisListType
    OP = mybir.AluOpType
    AF = mybir.ActivationFunctionType

    nc = bacc.Bacc(
        "TRN2", target_bir_lowering=False, debug=False,
        num_swdge_queues=nqueues,
        # The SWDGE descriptor-ring carveout scales with this (default 16KB
        # ~= 1024 descs, which serializes 896-desc gather calls: the ucode's
        # await_space blocks the Pool engine until the previous call
        # drains). A bigger carveout lets whole-tile gather calls (2560-2688
        # descs) queue up and drain back-to-back across the 4 queues.
        dynamic_dma_scratch_size=int(os.environ.get('BASS_SCRATCH', '65536')),
    )

    ctx_tab = nc.dram_tensor(
        "ctx_tab", [HALVES * CTX_CAP, EMB], f32, kind="ExternalInput"
    )
    out_tab = nc.dram_tensor(
        "out_tab", [HALVES * NG_CAP, EMB], f32, kind="ExternalInput"
    )
    ctx_widx = nc.dram_tensor(
        "ctx_widx", [P, tiles, CTX_W], i16, kind="ExternalInput"
    )
    ng_widx = nc.dram_tensor(
        "ng_widx", [P, tiles, NG_W], i16, kind="ExternalInput"
    )
    ident_in = nc.dram_tensor("ident", [P, P], bf16, kind="ExternalInput")
    partial = nc.dram_tensor("partial", [1, 1], f32, kind="ExternalOutput")

    with tile.TileContext(nc) as tc:
        with (
            tc.tile_pool(name="const", bufs=1) as cpool,
            tc.tile_pool(name="gather", bufs=6) as gpool,
            tc.tile_pool(name="work", bufs=2) as wpool,
            tc.tile_pool(name="psum", bufs=2, space=bass.MemorySpace.PSUM) as ppool,
        ):
            nc.gpsimd.load_library(library_config.mlp)

            ctx_widx_sb = cpool.tile([P, tiles, CTX_W], i16)
            nc.sync.dma_start(out=ctx_widx_sb[:], in_=ctx_widx[:])
            ng_widx_sb = cpool.tile([P, tiles, NG_W], i16)
            nc.sync.dma_start(out=ng_widx_sb[:], in_=ng_widx[:])

            ones = cpool.tile([P, 1], f32)
            nc.vector.memset(ones[:], 1.0)
            # Dummy Ln so the activation-table pass picks the set that
            # holds BOTH Ln and Exp up front — otherwise an Exp-only set
            # is loaded first and a 1.3us ACT_TABLE_LOAD lands on the
            # critical tail path right before the final Ln.
            warm = cpool.tile([P, 1], f32)
            nc.scalar.activation(out=warm[:], in_=ones[:], func=AF.Ln)
            ident = cpool.tile([P, P], bf16)
            nc.sync.dma_start(out=ident[:], in_=ident_in[:])
            exp_all = cpool.tile([P, tiles, N1], f32)

            # dma_gather descriptor-ring capacity caps one call at ~1024
            # indices (HW-measured); split each tile's gather into
            # <=CHUNK-slot calls.
            CHUNK = int(os.environ.get('BASS_CHUNK', '7'))  # slots per call
            # single_packet=True aggregates each DMA engine's descriptors
            # for a call into one SDMA packet, which caps a call at 64KB
            # per engine (2048 idxs at 512B rows). Calls bigger than that
            # need single_packet=False.
            SP = os.environ.get('BASS_SP', '1') == '1'

            # prepare_only pipelining: a plain dma_gather occupies its SWDGE
            # queue for descriptor-gen AND the full transfer drain (mailbox
            # depth 1), stalling the in-order Pool engine each round-robin
            # revisit. prepare_only decouples them: the queue is busy only
            # during gen; trigger_dma fires the ring and the transfer drains
            # in the background while later calls' descriptors generate.
            #
            # Each tile's ctx gather group and ng gather group are pinned to
            # ONE queue; two tiles' 4 groups cover the 4 queues, and their
            # calls are emitted round-robin so all four Q7 pairs generate
            # descriptors concurrently. Per-engine sub-queues are FIFO
            # within a queue, so one wait on a group's LAST call's
            # completion sem covers the whole group for consumers.
            #
            # Ring capacity is ~1024 descs/queue, so keep <=2 calls (of
            # <=512 idxs each) in the ring: before call r on a queue, wait
            # for call r-2 to drain. Tile's automatic consumer sync does
            # not understand user-sem'd preps (it waits lane sems the prep
            # never bumps — a real race caught by CoreSim), so consumers
            # wait the group sems explicitly on their own engine.
            # hybrid: even tiles gather via PLAIN calls (descriptor-gen on
            # the SWDGE queues' Q7 pairs, one call per queue in flight,
            # drain-serialized ~7.7us/call/queue) interleaved 1:1 with
            # prepare_only calls for odd tiles (descriptor-gen inline on the
            # otherwise-idle Pool engine, ~2.3us/call, drains ride the same
            # 16 DMA engines). Two generators run concurrently, so the DMA
            # engines stay fed instead of idling during gen windows.
            MODE = os.environ.get('BASS_MODE', 'hybrid')
            _st = os.environ.get('BASS_STAGGER', '')
            STAGGER = [int(x) for x in _st.split(',')] if _st else None
            # Two sems per queue (call-parity): a sem only ever has ONE
            # in-flight call incrementing it, so its 16 per-engine
            # completion increments can't interleave with another call's
            # and "sem >= 16*k" really means k calls of that parity done.
            dma_sems = [
                [nc.alloc_semaphore(f"gsem{q}_{p}") for p in range(2)]
                for q in range(nqueues)
            ]
            q_calls = [0] * nqueues
            call_idx = [0]  # PREP=0 fallback: round-robin queues per call

            def group_calls(out_tile, tab_ap, widx_sb, t, cols, splits=None):
                calls = []
                c0 = 0
                si = 0
                while c0 < cols:
                    step = splits[si] if splits and si < len(splits) else CHUNK
                    c1 = min(c0 + step, cols)
                    n = P * (c1 - c0)
                    calls.append(dict(
                        out_ap=out_tile[:, c0:c1, :],
                        in_ap=tab_ap,
                        idxs_ap=widx_sb[:, t, c0 * (P // 16) : c1 * (P // 16)],
                        num_idxs=n,
                        num_idxs_reg=n,
                        elem_size=EMB,
                        single_packet=SP,
                    ))
                    c0 = c1
                    si += 1
                return calls

            def emit_streams(streams):
                """Emit queue streams as rounds: ring-space waits, then one
                prep per queue (gens run concurrently), then triggers."""
                done = {}  # q -> (sem, value) completion of q's last call
                idx = [0] * nqueues
                while True:
                    rnd = [q for q in range(nqueues) if idx[q] < len(streams[q])]
                    if not rnd:
                        return done
                    for q in rnd:
                        r = q_calls[q]
                        prep = nc.gpsimd.dma_gather(
                            prepare_only=True, sem=dma_sems[q][r % 2],
                            queue_num=q, **streams[q][idx[q]]
                        )
                        if r >= 2:
                            # call r-2 (same parity) drained -> ring space.
                            # Attached to the prep itself: standalone
                            # EventSemaphores float under Tile's scheduler.
                            prep.wait_op(
                                dma_sems[q][r % 2], 16 * (r // 2), "sem-ge"
                            )
                        q_calls[q] += 1
                        idx[q] += 1
                        # all calls on q so far done == both parity sems at
                        # their cumulative counts (no cross-call ordering
                        # assumption needed)
                        done[q] = [
                            (dma_sems[q][p], 16 * ((q_calls[q] + 1 - p) // 2))
                            for p in range(2)
                            if (q_calls[q] + 1 - p) // 2 > 0
                        ]
                    for q in rnd:
                        nc.gpsimd.trigger_dma(count=None, queue_num=q)

            def emit_prep(q, kw):
                r = q_calls[q]
                prep = nc.gpsimd.dma_gather(
                    prepare_only=True, sem=dma_sems[q][r % 2],
                    queue_num=q, **kw
                )
                if r >= 2:
                    # call r-2 (same parity) drained -> ring space. Attached
                    # to the prep itself: standalone EventSemaphores float
                    # under Tile's scheduler.
                    prep.wait_op(dma_sems[q][r % 2], 16 * (r // 2), "sem-ge")
                nc.gpsimd.trigger_dma(count=None, queue_num=q)
                q_calls[q] += 1

            def q_done(q):
                # all prep calls on q so far done == both parity sems at
                # their cumulative counts (no cross-call ordering
                # assumption needed)
                return [
                    (dma_sems[q][p], 16 * ((q_calls[q] + 1 - p) // 2))
                    for p in range(2)
                    if (q_calls[q] + 1 - p) // 2 > 0
                ]

            def is_prep(t):
                if MODE == 'prep':
                    return True
                if MODE == 'hybrid':
                    return t % 2 == 1
                return False

            prev_reduce = None
            for pt in range(0, tiles, 2):
                pair = [t for t in (pt, pt + 1) if t < tiles]
                gtiles, gwaits = {}, {}
                streams = [[] for _ in range(nqueues)]
                plain_q, prep_q = [], []  # (kw,) / (q, kw, group_key)
                rot = pt // 2
                for i, t in enumerate(pair):
                    h = t // TPH
                    ctx_g = gpool.tile([P, C, EMB], f32, tag="ctx_g")
                    ng_g = gpool.tile([P, N1, EMB], f32, tag="ng_g")
                    gtiles[t] = (ctx_g, ng_g)
                    splits = None
                    if MODE == 'plain' and t == 0 and STAGGER:
                        splits = STAGGER
                    ctx_calls = group_calls(
                        ctx_g, ctx_tab[h * CTX_CAP : (h + 1) * CTX_CAP, :],
                        ctx_widx_sb, t, C, splits=splits,
                    )
                    ng_calls = group_calls(
                        ng_g, out_tab[h * NG_CAP : (h + 1) * NG_CAP, :],
                        ng_widx_sb, t, N1,
                    )
                    if MODE == 'prep':
                        qc = (2 * i + rot) % nqueues
                        qn = (2 * i + 1 + rot) % nqueues
                        streams[qc] = ctx_calls
                        streams[qn] = ng_calls
                        gwaits[t] = (qc, qn)
                    elif is_prep(t):
                        # group pinned to one queue so the consumer's two
                        # parity-sem waits cover all its calls
                        qc = rot % nqueues
                        qn = (rot + 1) % nqueues
                        prep_q += [(qc, kw, (t, 0)) for kw in ctx_calls]
                        prep_q += [(qn, kw, (t, 1)) for kw in ng_calls]
                        gwaits[t] = (qc, qn)
                    else:
                        plain_q += ctx_calls + ng_calls
                if MODE == 'prep':
                    done = emit_streams(streams)
                else:
                    # interleave plain and prep 1:1: a plain call re-visits
                    # its queue only every 4th plain slot, and the prep gens
                    # in between keep Pool useful instead of blocked on the
                    # plain mailbox
                    done = {}
                    for i in range(max(len(plain_q), len(prep_q))):
                        if i < len(plain_q):
                            nc.gpsimd.dma_gather(
                                queue_num=call_idx[0] % nqueues,
                                **plain_q[i]
                            )
                            call_idx[0] += 1
                        if i < len(prep_q):
                            q, kw, _ = prep_q[i]
                            emit_prep(q, kw)
                    for t in pair:
                        if is_prep(t):
                            done[gwaits[t][0]] = q_done(gwaits[t][0])
                            done[gwaits[t][1]] = q_done(gwaits[t][1])

                for t in pair:
                    ctx_g, ng_g = gtiles[t]
                    ctx_bf = wpool.tile([P, C, EMB], bf16, tag="ctx_bf")
                    act_i = nc.scalar.activation(
                        out=ctx_bf[:], in_=ctx_g[:], func=AF.Copy
                    )
                    if is_prep(t):
                        # one EventSemaphore holds both parity waits; the
                        # no-sync dep pins it before the consumer on the
                        # in-order engine (instructions only fit ~1 extra
                        # wait, and standalone waits float otherwise)
                        (s0, v0), *rest = done[gwaits[t][0]]
                        evt = nc.scalar.wait_ge(s0, v0)
                        for sem, val in rest:
                            evt.wait_op(sem, val, "sem-ge")
                        tile.add_dep_helper(
                            act_i.ins, evt.ins, sync=False,
                            reason="ctx gather group landed",
                        )
                    ctx_sum = ppool.tile([P, EMB], f32, tag="ctx_sum")
                    for c in range(C):
                        nc.tensor.matmul(
                            out=ctx_sum[:],
                            lhsT=ident[:],
                            rhs=ctx_bf[:, c, :],
                            start=(c == 0),
                            stop=(c == C - 1),
                        )

                    prod = wpool.tile([P, N1, EMB], bf16, tag="prod")
                    mult_i = nc.vector.tensor_tensor(
                        out=prod[:],
                        in0=ng_g[:],
                        in1=ctx_sum[:].unsqueeze(1).broadcast_to([P, N1, EMB]),
                        op=OP.mult,
                    )
                    if is_prep(t):
                        (s0, v0), *rest = done[gwaits[t][1]]
                        evt = nc.vector.wait_ge(s0, v0)
                        for sem, val in rest:
                            evt.wait_op(sem, val, "sem-ge")
                        tile.add_dep_helper(
                            mult_i.ins, evt.ins, sync=False,
                            reason="ng gather group landed",
                        )
                    if prev_reduce is not None:
                        # keep per-tile DVE order: reduce(t-1) before
                        # mult(t), else the scheduler defers reduces to the
                        # tail
                        tile.add_dep_helper(
                            mult_i.ins, prev_reduce.ins, sync=False,
                            reason="per-tile DVE order",
                        )
                    scores = wpool.tile([P, N1], f32, tag="scores")
                    prev_reduce = nc.vector.tensor_reduce(
                        out=scores[:], in_=prod[:], axis=AX.X, op=OP.add
                    )

                    clipped = wpool.tile([P, N1], f32, tag="clipped")
                    nc.vector.tensor_scalar(
                        out=clipped[:],
                        in0=scores[:],
                        scalar1=10.0,
                        scalar2=-10.0,
                        op0=OP.min,
                        op1=OP.max,
                    )

                    nc.scalar.activation(
                        out=exp_all[:, t, 0:N],
                        in_=clipped[:, 0:N],
                        func=AF.Exp,
                    )
                    nc.scalar.activation(
                        out=exp_all[:, t, N:N1],
                        in_=clipped[:, N:N1],
                        func=AF.Exp,
                        scale=-1.0,
                    )
            # softplus = ln(1 + exp(x)); accum_out sums all tiles*N1
            # softplus terms per partition in the same pass.
            ln_all = wpool.tile([P, tiles * N1], f32, tag="ln_all")
            tot = wpool.tile([P, 1], f32, tag="tot")
            nc.scalar.activation(
                out=ln_all[:],
                in_=exp_all[:].rearrange("p t c -> p (t c)"),
                func=AF.Ln,
                bias=1.0,
                accum_out=tot[:],
            )
            ps = ppool.tile([1, 1], f32, tag="ps")
            nc.tensor.matmul(
                out=ps[:], lhsT=ones[:], rhs=tot[:], start=True, stop=True
            )
            res = wpool.tile([1, 1], f32, tag="res")
            nc.vector.tensor_copy(out=res[:], in_=ps[:])
            nc.sync.dma_start(out=partial[:], in_=res[:])

    nc.compile()
    return nc


def _wrap_idx(inv_blk):
    """[128, cols] relabeled per-(partition, slot) ids -> dma_gather's
    wrapped [128, P*cols/16] int16 layout (idx list position i = j*128+p,
    wrapped W[q, s] = L[s*16+q], replicated across the 8 groups of 16
    partitions)."""
    L = inv_blk.T.reshape(-1)  # L[j*128 + p]
    W = L.reshape(-1, 16).T  # [16, n/16]
    return np.tile(W, (8, 1)).astype(np.int16)


def _prep_core(ctxi, ngi, ctx_tab, out_tab):
    """Per-core host prep: dedup+relabel per half per table; build the
    compacted table slabs and wrapped index tiles."""
    ctx_tab_u = np.zeros((HALVES * CTX_CAP, EMB), np.float32)
    out_tab_u = np.zeros((HALVES * NG_CAP, EMB), np.float32)
    ctx_w = np.empty((P, TILES, CTX_W), np.int16)
    ng_w = np.empty((P, TILES, NG_W), np.int16)
    rph = TPH * P  # rows per half
    for h in range(HALVES):
        rows = slice(h * rph, (h + 1) * rph)
        for idx, cap, tab, tab_u, w, cols in (
            (ctxi[rows], CTX_CAP, ctx_tab, ctx_tab_u, ctx_w, C),
            (ngi[rows], NG_CAP, out_tab, out_tab_u, ng_w, N1),
        ):
            uniq, inv = np.unique(idx, return_inverse=True)
            assert len(uniq) <= cap
            tab_u[h * cap : h * cap + len(uniq)] = tab[uniq]
            inv = inv.reshape(rph, cols)
            for tt in range(TPH):
                t = h * TPH + tt
                w[:, t, :] = _wrap_idx(inv[tt * P : (tt + 1) * P])
    return ctx_tab_u, out_tab_u, ctx_w, ng_w


def _prep_in_maps(inputs):
    pos_target = np.asarray(inputs["pos_target"]).astype(np.int64).reshape(B)
    pos_contexts = (
        np.asarray(inputs["pos_contexts"]).astype(np.int64).reshape(B, C)
    )
    pos_negatives = (
        np.asarray(inputs["pos_negatives"]).astype(np.int64).reshape(B, N)
    )
    ctx_tab = np.ascontiguousarray(
        np.asarray(inputs["context_table"], dtype=np.float32)
    )
    out_tab = np.ascontiguousarray(
        np.asarray(inputs["output_table"], dtype=np.float32)
    )
    ng = np.concatenate([pos_negatives, pos_target[:, None]], axis=1)

    in_maps = []
    for i in range(NCORES):
        sl = slice(i * RPC, (i + 1) * RPC)
        ctx_tab_u, out_tab_u, ctx_w, ng_w = _prep_core(
            pos_contexts[sl], ng[sl], ctx_tab, out_tab
        )
        in_maps.append(
            {
                "ctx_tab": ctx_tab_u,
                "out_tab": out_tab_u,
                "ctx_widx": ctx_w,
                "ng_widx": ng_w,
                "ident": _IDENT,
            }
        )
    return in_maps


def kernel(**inputs) -> np.ndarray:
    global _compiled, last_results
    if _compiled is None:
        _compiled = _build()
    nc = _compiled

    from concourse.bass_utils import run_bass_kernel_spmd

    in_maps = _prep_in_maps(inputs)
    trace = os.environ.get("BASS_PROFILE", "") == "1"
    r = run_bass_kernel_spmd(nc, in_maps, list(range(NCORES)), trace=trace)
    last_results = r
    total = sum(float(r.results[i]["partial"][0, 0]) for i in range(NCORES))
    return np.asarray(total / B, dtype=np.float32)



# revision 33
# speedup vs baseline: 2.6531x; 2.6531x over previous
"""CBOW negative-sampling loss on 8 TRN2 NeuronCores.

Data-parallel: batch dim (16384) sharded 8 ways (2048 rows/core).

The gather (the memory-bound core of this problem) uses the bulk
InstDMAGatherAnt extended instruction (~900 rows per instruction,
rotated across the 4 SWDGE queues so all four Q7 core pairs generate
DMA descriptors in parallel) instead of per-row indirect DMAs, which
cost ~1us of serialized descriptor-generation per 128 rows.
dma_gather takes int16 indices (< 32768), but VOCAB=100000 — so the
host dedups and relabels each half-core's referenced rows into a
compacted table upload with static per-half slabs:

  - per core, per half (1024 batch rows), per table: the referenced
    vocab rows are uniqued (sorted) and relabeled 0..U-1; the table
    slab uploaded to the device is table[uniq] padded to a static cap
    (cap = the draw count, an absolute bound on U, ~20.5k < 32768).
  - gather indices are the relabeled ids, wrapped in dma_gather's
    [16, n/16] layout and replicated across all 128 partitions.

Per tile of 128 rows (one batch row per partition):
  - 3 dma_gathers: 20 context rows/partition -> ctx_g [128, 20, 128]
  - 3 dma_gathers: 20 negatives + 1 target   -> ng_g  [128, 21, 128]
  - ACT copies ctx_g to bf16 (exact-identity matmul inputs)
  - PE: 20 PSUM-accumulating bf16 identity matmuls -> ctx_sum (fp32)
  - DVE: broadcast-mult (in1 straight from PSUM) + reduce over EMB
    -> scores [128, 21]; clip to [-10, 10] in one chained min/max op
  - ACT Exp: negs (softplus(+s)) and target with scale=-1
    (softplus(-s) == -log_sigmoid(s)) into slices of exp_all
Final: one ACT Ln(1 + x) with accum_out over all 16*21 values (= sum
of softplus terms per partition), then a ones-vector matmul on the PE
reduces across partitions.  Host sums the 8 partials and divides by B.
"""

import os
import numpy as np

VOCAB, EMB = 100000, 128
B, C, N = 16384, 20, 20
NCORES = 8
RPC = B // NCORES  # 2048 rows per core
P = 128
TILES = RPC // P  # 16
N1 = N + 1  # negatives + target
HALVES = 2
TPH = TILES // HALVES  # tiles per half
CTX_CAP = TPH * P * C  # 20480 — absolute bound on unique rows per half
NG_CAP = TPH * P * N1  # 21504
CTX_W = P * C // 16  # 160 wrapped idx cols per tile
NG_W = P * N1 // 16  # 168

_compiled = None
last_results = None
import ml_dtypes as _mld

_IDENT = np.eye(P, dtype=_mld.bfloat16)


def _build(tiles=TILES, nqueues=4):
    import concourse.bacc as bacc
    import concourse.tile as tile
    from concourse import bass, library_config, mybir

    f32 = mybir.dt.float32
    bf16 = mybir.dt.bfloat16
    i16 = mybir.dt.int16
    AX = mybir.AxisListType
    OP = mybir.AluOpType
    AF = mybir.ActivationFunctionType

    nc = bacc.Bacc(
        "TRN2", target_bir_lowering=False, debug=False,
        num_swdge_queues=nqueues,
        # The SWDGE descriptor-ring carveout scales with this (default 16KB
        # ~= 1024 descs, which serializes 896-desc gather calls: the ucode's
        # await_space blocks the Pool engine until the previous call
        # drains). A bigger carveout lets whole-tile gather calls (2560-2688
        # descs) queue up and drain back-to-back across the 4 queues.
        dynamic_dma_scratch_size=int(os.environ.get('BASS_SCRATCH', '65536')),
    )

    ctx_tab = nc.dram_tensor(
        "ctx_tab", [HALVES * CTX_CAP, EMB], f32, kind="ExternalInput"
    )
    out_tab = nc.dram_tensor(
        "out_tab", [HALVES * NG_CAP, EMB], f32, kind="ExternalInput"
    )
    ctx_widx = nc.dram_tensor(
        "ctx_widx", [P, tiles, CTX_W], i16, kind="ExternalInput"
    )
    ng_widx = nc.dram_tensor(
        "ng_widx", [P, tiles, NG_W], i16, kind="ExternalInput"
    )
    ident_in = nc.dram_tensor("ident", [P, P], bf16, kind="ExternalInput")
    partial = nc.dram_tensor("partial", [1, 2], f32, kind="ExternalOutput")

    with tile.TileContext(nc) as tc:
        with (
            tc.tile_pool(name="const", bufs=1) as cpool,
            tc.tile_pool(name="gather", bufs=6) as gpool,
            tc.tile_pool(name="work", bufs=2) as wpool,
            tc.tile_pool(name="psum", bufs=2, space=bass.MemorySpace.PSUM) as ppool,
        ):
            nc.gpsimd.load_library(library_config.mlp)

            # Load indices in per-pair slices so the first tiles' gathers
            # only wait on their own slice, not the full 1.3MB widx upload.
            ctx_widx_sb = cpool.tile([P, tiles, CTX_W], i16)
            ng_widx_sb = cpool.tile([P, tiles, NG_W], i16)
            for s0 in range(0, tiles, 2):
                s1 = min(s0 + 2, tiles)
                nc.sync.dma_start(
                    out=ctx_widx_sb[:, s0:s1, :], in_=ctx_widx[:, s0:s1, :]
                )
                nc.sync.dma_start(
                    out=ng_widx_sb[:, s0:s1, :], in_=ng_widx[:, s0:s1, :]
                )

            ones = cpool.tile([P, 1], f32)
            nc.vector.memset(ones[:], 1.0)
            # Dummy Ln so the activation-table pass picks the set that
            # holds BOTH Ln and Exp up front — otherwise an Exp-only set
            # is loaded first and a 1.3us ACT_TABLE_LOAD lands on the
            # critical tail path right before the final Ln.
            warm = cpool.tile([P, 1], f32)
            nc.scalar.activation(out=warm[:], in_=ones[:], func=AF.Ln)
            ident = cpool.tile([P, P], bf16)
            nc.sync.dma_start(out=ident[:], in_=ident_in[:])
            exp_all = cpool.tile([P, tiles, N1], f32)

            # dma_gather descriptor-ring capacity caps one call at ~1024
            # indices (HW-measured); split each tile's gather into
            # <=CHUNK-slot calls.
            CHUNK = int(os.environ.get('BASS_CHUNK', '7'))  # slots per call
            # single_packet=True aggregates each DMA engine's descriptors
            # for a call into one SDMA packet, which caps a call at 64KB
            # per engine (2048 idxs at 512B rows). Calls bigger than that
            # need single_packet=False.
            SP = os.environ.get('BASS_SP', '1') == '1'

            # prepare_only pipelining: a plain dma_gather occupies its SWDGE
            # queue for descriptor-gen AND the full transfer drain (mailbox
            # depth 1), stalling the in-order Pool engine each round-robin
            # revisit. prepare_only decouples them: the queue is busy only
            # during gen; trigger_dma fires the ring and the transfer drains
            # in the background while later calls' descriptors generate.
            #
            # Each tile's ctx gather group and ng gather group are pinned to
            # ONE queue; two tiles' 4 groups cover the 4 queues, and their
            # calls are emitted round-robin so all four Q7 pairs generate
            # descriptors concurrently. Per-engine sub-queues are FIFO
            # within a queue, so one wait on a group's LAST call's
            # completion sem covers the whole group for consumers.
            #
            # Ring capacity is ~1024 descs/queue, so keep <=2 calls (of
            # <=512 idxs each) in the ring: before call r on a queue, wait
            # for call r-2 to drain. Tile's automatic consumer sync does
            # not understand user-sem'd preps (it waits lane sems the prep
            # never bumps — a real race caught by CoreSim), so consumers
            # wait the group sems explicitly on their own engine.
            # hybrid: even tiles gather via PLAIN calls (descriptor-gen on
            # the SWDGE queues' Q7 pairs, one call per queue in flight,
            # drain-serialized ~7.7us/call/queue) interleaved 1:1 with
            # prepare_only calls for odd tiles (descriptor-gen inline on the
            # otherwise-idle Pool engine, ~2.3us/call, drains ride the same
            # 16 DMA engines). Two generators run concurrently, so the DMA
            # engines stay fed instead of idling during gen windows.
            MODE = os.environ.get('BASS_MODE', 'plain')
            _st = os.environ.get('BASS_STAGGER', '')
            STAGGER = [int(x) for x in _st.split(',')] if _st else None
            # Two sems per queue (call-parity): a sem only ever has ONE
            # in-flight call incrementing it, so its 16 per-engine
            # completion increments can't interleave with another call's
            # and "sem >= 16*k" really means k calls of that parity done.
            dma_sems = [
                [nc.alloc_semaphore(f"gsem{q}_{p}") for p in range(2)]
                for q in range(nqueues)
            ]
            q_calls = [0] * nqueues
            call_idx = [0]  # PREP=0 fallback: round-robin queues per call

            def group_calls(out_tile, tab_ap, widx_sb, t, cols, splits=None):
                calls = []
                c0 = 0
                si = 0
                while c0 < cols:
                    step = splits[si] if splits and si < len(splits) else CHUNK
                    c1 = min(c0 + step, cols)
                    n = P * (c1 - c0)
                    calls.append(dict(
                        out_ap=out_tile[:, c0:c1, :],
                        in_ap=tab_ap,
                        idxs_ap=widx_sb[:, t, c0 * (P // 16) : c1 * (P // 16)],
                        num_idxs=n,
                        num_idxs_reg=n,
                        elem_size=EMB,
                        single_packet=SP,
                    ))
                    c0 = c1
                    si += 1
                return calls

            def emit_streams(streams):
                """Emit queue streams as rounds: ring-space waits, then one
                prep per queue (gens run concurrently), then triggers."""
                done = {}  # q -> (sem, value) completion of q's last call
                idx = [0] * nqueues
                while True:
                    rnd = [q for q in range(nqueues) if idx[q] < len(streams[q])]
                    if not rnd:
                        return done
                    for q in rnd:
                        r = q_calls[q]
                        prep = nc.gpsimd.dma_gather(
                            prepare_only=True, sem=dma_sems[q][r % 2],
                            queue_num=q, **streams[q][idx[q]]
                        )
                        if r >= 2:
                            # call r-2 (same parity) drained -> ring space.
                            # Attached to the prep itself: standalone
                            # EventSemaphores float under Tile's scheduler.
                            prep.wait_op(
                                dma_sems[q][r % 2], 16 * (r // 2), "sem-ge"
                            )
                        q_calls[q] += 1
                        idx[q] += 1
                        # all calls on q so far done == both parity sems at
                        # their cumulative counts (no cross-call ordering
                        # assumption needed)
                        done[q] = [
                            (dma_sems[q][p], 16 * ((q_calls[q] + 1 - p) // 2))
                            for p in range(2)
                            if (q_calls[q] + 1 - p) // 2 > 0
                        ]
                    for q in rnd:
                        nc.gpsimd.trigger_dma(count=None, queue_num=q)

            def emit_prep(q, kw):
                r = q_calls[q]
                prep = nc.gpsimd.dma_gather(
                    prepare_only=True, sem=dma_sems[q][r % 2],
                    queue_num=q, **kw
                )
                if r >= 2:
                    # call r-2 (same parity) drained -> ring space. Attached
                    # to the prep itself: standalone EventSemaphores float
                    # under Tile's scheduler.
                    prep.wait_op(dma_sems[q][r % 2], 16 * (r // 2), "sem-ge")
                nc.gpsimd.trigger_dma(count=None, queue_num=q)
                q_calls[q] += 1

            def q_done(q):
                # all prep calls on q so far done == both parity sems at
                # their cumulative counts (no cross-call ordering
                # assumption needed)
                return [
                    (dma_sems[q][p], 16 * ((q_calls[q] + 1 - p) // 2))
                    for p in range(2)
                    if (q_calls[q] + 1 - p) // 2 > 0
                ]

            def is_prep(t):
                if MODE == 'prep':
                    return True
                if MODE == 'hybrid':
                    return t % 2 == 1
                return False

            prev_reduce = None
            for pt in range(0, tiles, 2):
                pair = [t for t in (pt, pt + 1) if t < tiles]
                gtiles, gwaits = {}, {}
                streams = [[] for _ in range(nqueues)]
                plain_q, prep_q = [], []  # (kw,) / (q, kw, group_key)
                rot = pt // 2
                for i, t in enumerate(pair):
                    h = t // TPH
                    ctx_g = gpool.tile([P, C, EMB], f32, tag="ctx_g")
                    ng_g = gpool.tile([P, N1, EMB], f32, tag="ng_g")
                    gtiles[t] = (ctx_g, ng_g)
                    splits = None
                    if MODE == 'plain' and t == 0 and STAGGER:
                        splits = STAGGER
                    ctx_calls = group_calls(
                        ctx_g, ctx_tab[h * CTX_CAP : (h + 1) * CTX_CAP, :],
                        ctx_widx_sb, t, C, splits=splits,
                    )
                    ng_calls = group_calls(
                        ng_g, out_tab[h * NG_CAP : (h + 1) * NG_CAP, :],
                        ng_widx_sb, t, N1,
                    )
                    if MODE == 'prep':
                        qc = (2 * i + rot) % nqueues
                        qn = (2 * i + 1 + rot) % nqueues
                        streams[qc] = ctx_calls
                        streams[qn] = ng_calls
                        gwaits[t] = (qc, qn)
                    elif is_prep(t):
                        # group pinned to one queue so the consumer's two
                        # parity-sem waits cover all its calls
                        qc = rot % nqueues
                        qn = (rot + 1) % nqueues
                        prep_q += [(qc, kw, (t, 0)) for kw in ctx_calls]
                        prep_q += [(qn, kw, (t, 1)) for kw in ng_calls]
                        gwaits[t] = (qc, qn)
                    else:
                        plain_q += ctx_calls + ng_calls
                if MODE == 'prep':
                    done = emit_streams(streams)
                else:
                    # interleave plain and prep 1:1: a plain call re-visits
                    # its queue only every 4th plain slot, and the prep gens
                    # in between keep Pool useful instead of blocked on the
                    # plain mailbox
                    done = {}
                    for i in range(max(len(plain_q), len(prep_q))):
                        if i < len(plain_q):
                            nc.gpsimd.dma_gather(
                                queue_num=call_idx[0] % nqueues,
                                **plain_q[i]
                            )
                            call_idx[0] += 1
                        if i < len(prep_q):
                            q, kw, _ = prep_q[i]
                            emit_prep(q, kw)
                    for t in pair:
                        if is_prep(t):
                            done[gwaits[t][0]] = q_done(gwaits[t][0])
                            done[gwaits[t][1]] = q_done(gwaits[t][1])

                for t in pair:
                    ctx_g, ng_g = gtiles[t]
                    ctx_bf = wpool.tile([P, C, EMB], bf16, tag="ctx_bf")
                    act_i = nc.scalar.activation(
                        out=ctx_bf[:], in_=ctx_g[:], func=AF.Copy
                    )
                    if is_prep(t):
                        # one EventSemaphore holds both parity waits; the
                        # no-sync dep pins it before the consumer on the
                        # in-order engine (instructions only fit ~1 extra
                        # wait, and standalone waits float otherwise)
                        (s0, v0), *rest = done[gwaits[t][0]]
                        evt = nc.scalar.wait_ge(s0, v0)
                        for sem, val in rest:
                            evt.wait_op(sem, val, "sem-ge")
                        tile.add_dep_helper(
                            act_i.ins, evt.ins, sync=False,
                            reason="ctx gather group landed",
                        )
                    ctx_sum = ppool.tile([P, EMB], f32, tag="ctx_sum")
                    for c in range(C):
                        nc.tensor.matmul(
                            out=ctx_sum[:],
                            lhsT=ident[:],
                            rhs=ctx_bf[:, c, :],
                            start=(c == 0),
                            stop=(c == C - 1),
                        )

                    prod = wpool.tile([P, N1, EMB], bf16, tag="prod")
                    mult_i = nc.vector.tensor_tensor(
                        out=prod[:],
                        in0=ng_g[:],
                        in1=ctx_sum[:].unsqueeze(1).broadcast_to([P, N1, EMB]),
                        op=OP.mult,
                    )
                    if is_prep(t):
                        (s0, v0), *rest = done[gwaits[t][1]]
                        evt = nc.vector.wait_ge(s0, v0)
                        for sem, val in rest:
                            evt.wait_op(sem, val, "sem-ge")
                        tile.add_dep_helper(
                            mult_i.ins, evt.ins, sync=False,
                            reason="ng gather group landed",
                        )
                    if prev_reduce is not None:
                        # keep per-tile DVE order: reduce(t-1) before
                        # mult(t), else the scheduler defers reduces to the
                        # tail
                        tile.add_dep_helper(
                            mult_i.ins, prev_reduce.ins, sync=False,
                            reason="per-tile DVE order",
                        )
                    scores = wpool.tile([P, N1], f32, tag="scores")
                    prev_reduce = nc.vector.tensor_reduce(
                        out=scores[:], in_=prod[:], axis=AX.X, op=OP.add
                    )

                    clipped = wpool.tile([P, N1], f32, tag="clipped")
                    nc.vector.tensor_scalar(
                        out=clipped[:],
                        in0=scores[:],
                        scalar1=10.0,
                        scalar2=-10.0,
                        op0=OP.min,
                        op1=OP.max,
                    )

                    nc.scalar.activation(
                        out=exp_all[:, t, 0:N],
                        in_=clipped[:, 0:N],
                        func=AF.Exp,
                    )
                    nc.scalar.activation(
                        out=exp_all[:, t, N:N1],
                        in_=clipped[:, N:N1],
                        func=AF.Exp,
                        scale=-1.0,
                    )
            # softplus = ln(1 + exp(x)); accum_out sums the softplus terms
            # per partition. Two passes: the bulk (tiles 0..T-3) runs as
            # soon as those tiles' exps land, overlapping the last tiles'
            # gathers; only the small second pass sits on the tail.
            tsplit = max(tiles - 2, 1)
            ln_all = wpool.tile([P, tiles * N1], f32, tag="ln_all")
            tots = wpool.tile([P, 2], f32, tag="tots")
            flat = exp_all[:].rearrange("p t c -> p (t c)")
            nc.scalar.activation(
                out=ln_all[:, : tsplit * N1],
                in_=flat[:, : tsplit * N1],
                func=AF.Ln,
                bias=1.0,
                accum_out=tots[:, 0:1],
            )
            nc.scalar.activation(
                out=ln_all[:, tsplit * N1 :],
                in_=flat[:, tsplit * N1 :],
                func=AF.Ln,
                bias=1.0,
                accum_out=tots[:, 1:2],
            )
            ps = ppool.tile([1, 2], f32, tag="ps")
            nc.tensor.matmul(
                out=ps[:], lhsT=ones[:], rhs=tots[:], start=True, stop=True
            )
            res = wpool.tile([1, 2], f32, tag="res")
            nc.vector.tensor_copy(out=res[:], in_=ps[:])
            nc.sync.dma_start(out=partial[:], in_=res[:])

    nc.compile()
    return nc


def _wrap_idx(inv_blk):
    """[128, cols] relabeled per-(partition, slot) ids -> dma_gather's
    wrapped [128, P*cols/16] int16 layout (idx list position i = j*128+p,
    wrapped W[q, s] = L[s*16+q], replicated across the 8 groups of 16
    partitions)."""
    L = inv_blk.T.reshape(-1)  # L[j*128 + p]
    W = L.reshape(-1, 16).T  # [16, n/16]
    return np.tile(W, (8, 1)).astype(np.int16)


def _prep_core(ctxi, ngi, ctx_tab, out_tab):
    """Per-core host prep: dedup+relabel per half per table; build the
    compacted table slabs and wrapped index tiles."""
    ctx_tab_u = np.zeros((HALVES * CTX_CAP, EMB), np.float32)
    out_tab_u = np.zeros((HALVES * NG_CAP, EMB), np.float32)
    ctx_w = np.empty((P, TILES, CTX_W), np.int16)
    ng_w = np.empty((P, TILES, NG_W), np.int16)
    rph = TPH * P  # rows per half
    for h in range(HALVES):
        rows = slice(h * rph, (h + 1) * rph)
        for idx, cap, tab, tab_u, w, cols in (
            (ctxi[rows], CTX_CAP, ctx_tab, ctx_tab_u, ctx_w, C),
            (ngi[rows], NG_CAP, out_tab, out_tab_u, ng_w, N1),
        ):
            uniq, inv = np.unique(idx, return_inverse=True)
            assert len(uniq) <= cap
            tab_u[h * cap : h * cap + len(uniq)] = tab[uniq]
            inv = inv.reshape(rph, cols)
            for tt in range(TPH):
                t = h * TPH + tt
                w[:, t, :] = _wrap_idx(inv[tt * P : (tt + 1) * P])
    return ctx_tab_u, out_tab_u, ctx_w, ng_w


def _prep_in_maps(inputs):
    pos_target = np.asarray(inputs["pos_target"]).astype(np.int64).reshape(B)
    pos_contexts = (
        np.asarray(inputs["pos_contexts"]).astype(np.int64).reshape(B, C)
    )
    pos_negatives = (
        np.asarray(inputs["pos_negatives"]).astype(np.int64).reshape(B, N)
    )
    ctx_tab = np.ascontiguousarray(
        np.asarray(inputs["context_table"], dtype=np.float32)
    )
    out_tab = np.ascontiguousarray(
        np.asarray(inputs["output_table"], dtype=np.float32)
    )
    ng = np.concatenate([pos_negatives, pos_target[:, None]], axis=1)

    in_maps = []
    for i in range(NCORES):
        sl = slice(i * RPC, (i + 1) * RPC)
        ctx_tab_u, out_tab_u, ctx_w, ng_w = _prep_core(
            pos_contexts[sl], ng[sl], ctx_tab, out_tab
        )
        in_maps.append(
            {
                "ctx_tab": ctx_tab_u,
                "out_tab": out_tab_u,
                "ctx_widx": ctx_w,
                "ng_widx": ng_w,
                "ident": _IDENT,
            }
        )
    return in_maps


def kernel(**inputs) -> np.ndarray:
    global _compiled, last_results
    if _compiled is None:
        _compiled = _build()
    nc = _compiled

    from concourse.bass_utils import run_bass_kernel_spmd

    in_maps = _prep_in_maps(inputs)
    trace = os.environ.get("BASS_PROFILE", "") == "1"
    r = run_bass_kernel_spmd(nc, in_maps, list(range(NCORES)), trace=trace)
    last_results = r
    total = sum(float(np.sum(r.results[i]["partial"])) for i in range(NCORES))
    return np.asarray(total / B, dtype=np.float32)

